# revision 14
# baseline (speedup 1.0000x reference)
"""Trainium2 Bass kernel: multi-head encoder-decoder attention.

Problem: B=4, S=2048, E=512, H=8, DH=64 (f32 reference).

Sharding: (batch, query-half) -> 8 cores. Core c = 2*b + half computes
all 8 heads of attention for query rows [half*1024, (half+1)*1024) of
batch b against the batch's full K/V, plus the output projection for
those rows. No cross-core communication; the host concatenates the
eight [E, 1024] outputs (transposed) into [B, S, E].

Per-core layout strategy (all activations fp16, PSUM accumulation f32):
  - Host pre-transposes embeddings to [E, seq] so projections need no
    on-chip transposes: Q^T/K^T per head are produced directly as
    [DH, seq] (scale 1/sqrt(DH) and bQ folded into WQ/bQ host-side).
  - scores^T[k, q] = K'^T.T @ Q'^T per 128-row k-tile -> PSUM strip
    [128, 1024]; exp on ScalarE (no max subtraction: |scores| <~ 2.5,
    mathematically exact softmax) -> fp16 SBUF strip.
  - ctx^T accumulation: lhsT = [V_h | ones] [128, 65], rhs = exp strip;
    row 64 of the accumulator is then the softmax denominator for free.
  - normalize: reciprocal of denom row, partition-broadcast, multiply,
    add bV (exact: sum(attn)=1), write into cat^T tiles.
  - out^T = WO @ cat^T (+ bO) -> DRAM [E, 1024] f32.
"""

import sys

import numpy as np

try:
    import concourse.bass as bass  # noqa: F401
except ImportError:  # fresh grading dir: fall back to the repo path
    sys.path.insert(0, "/opt/trn_rl_repo")

B, S, E, H, DH = 4, 2048, 512, 8, 64
NQ = S // 2  # 1024 query rows per core
ET = E // 128  # 4 E-tiles
KTN = S // 128  # 16 key tiles
NCORES = 8

_CACHE: dict = {}


def _build_nc(debug_taps=False):
    import concourse.tile as tile
    from concourse import bacc, mybir

    f16 = mybir.dt.float16
    f32 = mybir.dt.float32
    AF = mybir.ActivationFunctionType

    nc = bacc.Bacc(
        "TRN2", target_bir_lowering=False, debug=False, num_devices=NCORES
    )

    # DRAM parameters (per-core shards; host-packed layouts)
    xq = nc.dram_tensor("xq", [128, ET, NQ], f16, kind="ExternalInput").ap()
    xk = nc.dram_tensor("xk", [128, ET, S], f16, kind="ExternalInput").ap()
    xv = nc.dram_tensor("xv", [128, ET, S], f16, kind="ExternalInput").ap()
    wq = nc.dram_tensor("wq", [128, ET, 4, 128], f16, kind="ExternalInput").ap()
    wk = nc.dram_tensor("wk", [128, ET, 4, 128], f16, kind="ExternalInput").ap()
    wv = nc.dram_tensor("wv", [128, ET, 512], f16, kind="ExternalInput").ap()
    wo = nc.dram_tensor("wo", [128, ET, 512], f16, kind="ExternalInput").ap()
    bq = nc.dram_tensor("bq", [128, 4], f32, kind="ExternalInput").ap()
    bk = nc.dram_tensor("bk", [128, 4], f32, kind="ExternalInput").ap()
    bv = nc.dram_tensor("bv", [64, H], f32, kind="ExternalInput").ap()
    bo = nc.dram_tensor("bo", [128, ET], f32, kind="ExternalInput").ap()
    outT = nc.dram_tensor("outT", [E, NQ], f32, kind="ExternalOutput").ap()
    taps = {}
    if debug_taps:
        taps["dq0"] = nc.dram_tensor("dq0", [64, NQ], f16, kind="ExternalOutput").ap()
        taps["dk0"] = nc.dram_tensor("dk0", [64, S], f16, kind="ExternalOutput").ap()
        taps["dv0"] = nc.dram_tensor("dv0", [128, H * 65], f16, kind="ExternalOutput").ap()
        taps["dps"] = nc.dram_tensor("dps", [128, NQ], f32, kind="ExternalOutput").ap()
        taps["des"] = nc.dram_tensor("des", [128, NQ], f16, kind="ExternalOutput").ap()
        taps["dcx"] = nc.dram_tensor("dcx", [65, NQ], f32, kind="ExternalOutput").ap()
        for et in range(ET):
            taps[f"dcat{et}"] = nc.dram_tensor(
                f"dcat{et}", [128, NQ], f16, kind="ExternalOutput"
            ).ap()
        taps["drecip"] = nc.dram_tensor("drecip", [1, NQ], f32, kind="ExternalOutput").ap()
        taps["drrep"] = nc.dram_tensor("drrep", [64, NQ], f32, kind="ExternalOutput").ap()
        taps["dtmp"] = nc.dram_tensor("dtmp", [64, NQ], f32, kind="ExternalOutput").ap()

    with tile.TileContext(nc) as tc:
        with tc.tile_pool(name="const", bufs=1) as const, tc.tile_pool(
            name="work", bufs=1
        ) as work, tc.tile_pool(name="expp", bufs=4) as expp, tc.tile_pool(
            name="spsum", bufs=2, space="PSUM"
        ) as spsum, tc.tile_pool(name="cpsum", bufs=2, space="PSUM") as cpsum:
            # ---- input DMA (ordered so the Q/K projections can start early)
            wq_sb = const.tile([128, ET, 4, 128], f16, name="wq_sb", tag="wq")
            nc.sync.dma_start(wq_sb, wq)
            bq_sb = const.tile([128, 4], f32, name="bq_sb", tag="bq")
            nc.sync.dma_start(bq_sb, bq)
            xq_sb = const.tile([128, ET, NQ], f16, name="xq_sb", tag="xq")
            nc.sync.dma_start(xq_sb, xq)
            wk_sb = const.tile([128, ET, 4, 128], f16, name="wk_sb", tag="wk")
            nc.sync.dma_start(wk_sb, wk)
            bk_sb = const.tile([128, 4], f32, name="bk_sb", tag="bk")
            nc.sync.dma_start(bk_sb, bk)
            xk_sb = const.tile([128, ET, S], f16, name="xk_sb", tag="xk")
            nc.sync.dma_start(xk_sb, xk)
            wv_sb = const.tile([128, ET, 512], f16, name="wv_sb", tag="wv")
            nc.sync.dma_start(wv_sb, wv)
            xv_sb = const.tile([128, ET, S], f16, name="xv_sb", tag="xv")
            nc.sync.dma_start(xv_sb, xv)
            wo_sb = const.tile([128, ET, 512], f16, name="wo_sb", tag="wo")
            nc.sync.dma_start(wo_sb, wo)
            bv_sb = const.tile([64, H], f32, name="bv_sb", tag="bv")
            nc.sync.dma_start(bv_sb, bv)
            bo_sb = const.tile([128, ET], f32, name="bo_sb", tag="bo")
            nc.sync.dma_start(bo_sb, bo)

            # ---- persistent on-chip tensors
            qt = [
                work.tile([64, NQ], f16, name=f"qt{h}", tag=f"qt{h}")
                for h in range(H)
            ]
            kt = [
                work.tile([64, S], f16, name=f"kt{h}", tag=f"kt{h}")
                for h in range(H)
            ]
            vsb = [
                work.tile([128, H, 65], f16, name=f"v{st}", tag=f"v{st}")
                for st in range(KTN)
            ]
            catT = [
                work.tile([128, NQ], f16, name=f"cat{et}", tag=f"cat{et}")
                for et in range(ET)
            ]

            def emit_qt_pair(p):
                ps = spsum.tile([128, NQ], f32, name=f"qtp{p}", tag="strip")
                for qc in range(2):
                    for et in range(ET):
                        nc.tensor.matmul(
                            ps[:, qc * 512 : (qc + 1) * 512],
                            wq_sb[:, et, p, :],
                            xq_sb[:, et, qc * 512 : (qc + 1) * 512],
                            start=(et == 0),
                            stop=(et == ET - 1),
                        )
                nc.vector.tensor_scalar_add(
                    qt[2 * p], ps[0:64, :], bq_sb[0:64, p : p + 1]
                )
                nc.vector.tensor_scalar_add(
                    qt[2 * p + 1], ps[64:128, :], bq_sb[64:128, p : p + 1]
                )

            def emit_kt_pair(p, kc):
                ps = spsum.tile([128, NQ], f32, name=f"ktp{p}_{kc}", tag="strip")
                for n in range(2):
                    for et in range(ET):
                        nc.tensor.matmul(
                            ps[:, n * 512 : (n + 1) * 512],
                            wk_sb[:, et, p, :],
                            xk_sb[
                                :,
                                et,
                                kc * NQ + n * 512 : kc * NQ + (n + 1) * 512,
                            ],
                            start=(et == 0),
                            stop=(et == ET - 1),
                        )
                nc.vector.tensor_scalar_add(
                    kt[2 * p][:, kc * NQ : (kc + 1) * NQ],
                    ps[0:64, :],
                    bk_sb[0:64, p : p + 1],
                )
                nc.vector.tensor_scalar_add(
                    kt[2 * p + 1][:, kc * NQ : (kc + 1) * NQ],
                    ps[64:128, :],
                    bk_sb[64:128, p : p + 1],
                )

            def emit_v(st):
                ps = spsum.tile([128, 512], f32, name=f"vps{st}", tag="strip")
                for et in range(ET):
                    nc.tensor.matmul(
                        ps,
                        xv_sb[:, et, st * 128 : (st + 1) * 128],
                        wv_sb[:, et, :],
                        start=(et == 0),
                        stop=(et == ET - 1),
                    )
                nc.vector.tensor_copy(
                    vsb[st][:, :, 0:64], ps.rearrange("k (h d) -> k h d", h=H)
                )
                nc.vector.memset(vsb[st][:, :, 64:65], 1.0)

            def emit_head(h, extras):
                cx = cpsum.tile([65, NQ], f32, name=f"cx{h}", tag="ctx")
                for st in range(KTN):
                    if extras:
                        extras.pop(0)()
                    ps = spsum.tile(
                        [128, NQ], f32, name=f"sc{h}_{st}", tag="strip"
                    )
                    for qc in range(2):
                        nc.tensor.matmul(
                            ps[:, qc * 512 : (qc + 1) * 512],
                            kt[h][:, st * 128 : (st + 1) * 128],
                            qt[h][:, qc * 512 : (qc + 1) * 512],
                            start=True,
                            stop=True,
                        )
                    es = expp.tile([128, NQ], f16, name=f"es{h}_{st}", tag="exp")
                    nc.scalar.activation(es, ps, AF.Exp)
                    if taps and h == 0 and st == 0:
                        dpss = work.tile([128, NQ], f32, name="dpss", tag="dpss")
                        nc.vector.tensor_copy(dpss, ps)
                        nc.sync.dma_start(taps["dps"], dpss)
                        nc.sync.dma_start(taps["des"], es)
                    for qc in range(2):
                        nc.tensor.matmul(
                            cx[:, qc * 512 : (qc + 1) * 512],
                            vsb[st][:, h, :],
                            es[:, qc * 512 : (qc + 1) * 512],
                            start=(st == 0),
                            stop=(st == KTN - 1),
                        )
                # normalize + bV -> catT
                if taps and h == 0:
                    dcxs = work.tile([65, NQ], f32, name="dcxs", tag="dcxs")
                    nc.vector.tensor_copy(dcxs, cx)
                    nc.sync.dma_start(taps["dcx"], dcxs)
                # NB: reciprocal_approx_fast (custom DVE) drops PSUM partition
                # offsets on HW, and plain reciprocal traps to a slow software
                # handler — so copy the denominator row to SBUF partition 0
                # first, then run the fast approx there.
                srow = work.tile([1, NQ], f32, name=f"sr{h}", tag="srow", bufs=2)
                nc.vector.tensor_copy(srow, cx[64:65, :])
                recip = work.tile([1, NQ], f32, name=f"rc{h}", tag="recip", bufs=2)
                nc.vector.reciprocal_approx_fast(recip, srow)
                rrep = work.tile([64, NQ], f32, name=f"rr{h}", tag="rrep", bufs=2)
                nc.gpsimd.partition_broadcast(rrep, recip)
                tmp = work.tile([64, NQ], f32, name=f"tm{h}", tag="ctmp", bufs=2)
                nc.vector.tensor_mul(tmp, cx[0:64, :], rrep)
                if taps and h == 0:
                    nc.sync.dma_start(taps["drecip"], recip)
                    nc.sync.dma_start(taps["drrep"], rrep)
                    nc.sync.dma_start(taps["dtmp"], tmp)
                po = (h % 2) * 64
                nc.vector.tensor_scalar_add(
                    catT[h // 2][po : po + 64, :], tmp, bv_sb[:, h : h + 1]
                )

            # ---- emission schedule
            emit_qt_pair(0)
            emit_kt_pair(0, 0)
            emit_kt_pair(0, 1)
            emit_v(0)
            emit_v(1)

            extras_by_head: dict = {
                0: [(lambda st=st: emit_v(st)) for st in range(2, KTN)],
                1: [
                    lambda: emit_qt_pair(1),
                    lambda: emit_kt_pair(1, 0),
                    lambda: emit_kt_pair(1, 1),
                ],
                3: [
                    lambda: emit_qt_pair(2),
                    lambda: emit_kt_pair(2, 0),
                    lambda: emit_kt_pair(2, 1),
                ],
                5: [
                    lambda: emit_qt_pair(3),
                    lambda: emit_kt_pair(3, 0),
                    lambda: emit_kt_pair(3, 1),
                ],
            }
            for h in range(H):
                emit_head(h, extras_by_head.get(h, []))

            if taps:
                nc.sync.dma_start(taps["dq0"], qt[0])
                nc.sync.dma_start(taps["dk0"], kt[0])
                nc.sync.dma_start(
                    taps["dv0"], vsb[0].rearrange("k h d -> k (h d)")
                )
                for et in range(ET):
                    nc.sync.dma_start(taps[f"dcat{et}"], catT[et])

            # ---- output projection: out^T = WO @ cat^T + bO
            for eo in range(ET):
                ps = spsum.tile([128, NQ], f32, name=f"wops{eo}", tag="strip")
                for qc in range(2):
                    for et in range(ET):
                        nc.tensor.matmul(
                            ps[:, qc * 512 : (qc + 1) * 512],
                            wo_sb[:, et, eo * 128 : (eo + 1) * 128],
                            catT[et][:, qc * 512 : (qc + 1) * 512],
                            start=(et == 0),
                            stop=(et == ET - 1),
                        )
                osb = work.tile([128, NQ], f32, name=f"osb{eo}", tag="osb", bufs=2)
                nc.vector.tensor_scalar_add(osb, ps, bo_sb[:, eo : eo + 1])
                nc.sync.dma_start(outT[eo * 128 : (eo + 1) * 128, :], osb)

    nc.compile()
    return nc


def _pack_weights(WQ, bQ, WK, bK, WV, bV, WO, bO):
    """Host-side weight packing (layouts documented in _build_nc)."""
    scale = 1.0 / np.sqrt(DH)
    WQs = (np.asarray(WQ, np.float32) * scale).astype(np.float16)
    bQs = np.asarray(bQ, np.float32) * scale
    WKh = np.asarray(WK, np.float32).astype(np.float16)
    WVh = np.asarray(WV, np.float32).astype(np.float16)
    WOh = np.asarray(WO, np.float32).astype(np.float16)

    wq_p = np.zeros((128, ET, 4, 128), np.float16)
    wk_p = np.zeros((128, ET, 4, 128), np.float16)
    for p in range(4):
        for et in range(ET):
            sl = slice(et * 128, (et + 1) * 128)
            wq_p[:, et, p, 0:64] = WQs[2 * p, sl, :]
            wq_p[:, et, p, 64:128] = WQs[2 * p + 1, sl, :]
            wk_p[:, et, p, 0:64] = WKh[2 * p, sl, :]
            wk_p[:, et, p, 64:128] = WKh[2 * p + 1, sl, :]

    wv_p = np.zeros((128, ET, 512), np.float16)
    wo_p = np.zeros((128, ET, 512), np.float16)
    for et in range(ET):
        sl = slice(et * 128, (et + 1) * 128)
        for h in range(H):
            wv_p[:, et, 64 * h : 64 * h + 64] = WVh[h, sl, :]
        wo_p[:, et, :] = WOh[:, sl].T

    bq_p = np.zeros((128, 4), np.float32)
    bk_p = np.zeros((128, 4), np.float32)
    for p in range(4):
        bq_p[0:64, p] = bQs[2 * p]
        bq_p[64:128, p] = bQs[2 * p + 1]
        bk_p[0:64, p] = np.asarray(bK, np.float32)[2 * p]
        bk_p[64:128, p] = np.asarray(bK, np.float32)[2 * p + 1]

    bv_p = np.asarray(bV, np.float32).T.copy()  # [64, H]
    bo_p = np.asarray(bO, np.float32).reshape(ET, 128).T.copy()  # [128, ET]

    return dict(
        wq=wq_p, wk=wk_p, wv=wv_p, wo=wo_p, bq=bq_p, bk=bk_p, bv=bv_p, bo=bo_p
    )


def _pack_seq_T(x2d):
    """[seq, E] f32 -> [128, ET, seq] f16 (transposed, E-tiled)."""
    seq = x2d.shape[0]
    return (
        np.ascontiguousarray(x2d.T)
        .reshape(ET, 128, seq)
        .transpose(1, 0, 2)
        .astype(np.float16)
    )


def prepare(embeddings_q, embeddings_k, embeddings_v, WQ, bQ, WK, bK, WV, bV, WO, bO):
    """Build per-core input maps. Returns in_maps list of dicts."""
    w = _pack_weights(WQ, bQ, WK, bK, WV, bV, WO, bO)
    eq = np.asarray(embeddings_q, np.float32)
    ek = np.asarray(embeddings_k, np.float32)
    ev = np.asarray(embeddings_v, np.float32)
    in_maps = []
    for b in range(B):
        xk_p = _pack_seq_T(ek[b])
        xv_p = _pack_seq_T(ev[b])
        for half in range(2):
            xq_p = _pack_seq_T(eq[b, half * NQ : (half + 1) * NQ, :])
            m = {"xq": xq_p, "xk": xk_p, "xv": xv_p}
            m.update(w)
            in_maps.append(m)
    return in_maps


def get_nc():
    if "nc" not in _CACHE:
        _CACHE["nc"] = _build_nc()
    return _CACHE["nc"]


def assemble(core_outs):
    """core_outs: list of 8 dicts with 'outT' [E, NQ] f32 -> [B, S, E]."""
    out = np.empty((B, S, E), np.float32)
    for b in range(B):
        for half in range(2):
            c = 2 * b + half
            out[b, half * NQ : (half + 1) * NQ, :] = core_outs[c]["outT"].T
    return out


def kernel(**inputs):
    from concourse.bass_utils import run_bass_kernel_spmd

    nc = get_nc()
    in_maps = prepare(**inputs)
    res = run_bass_kernel_spmd(nc, in_maps, core_ids=list(range(NCORES)))
    return assemble(res.results)


# revision 16
# speedup vs baseline: 5.2467x; 5.2467x over previous
"""Trainium2 Bass kernel: multi-head encoder-decoder attention.

Problem: B=4, S=2048, E=512, H=8, DH=64 (f32 reference).

Sharding: (batch, query-half) -> 8 cores. Core c = 2*b + half computes
all 8 heads of attention for query rows [half*1024, (half+1)*1024) of
batch b against the batch's full K/V, plus the output projection for
those rows. No cross-core communication; the host concatenates the
eight [E, 1024] outputs (transposed) into [B, S, E].

Per-core layout strategy (all activations fp16, PSUM accumulation f32):
  - Host pre-transposes embeddings to [E, seq] so projections need no
    on-chip transposes: Q^T/K^T per head are produced directly as
    [DH, seq] (scale 1/sqrt(DH) and bQ folded into WQ/bQ host-side).
  - scores^T[k, q] = K'^T.T @ Q'^T per 128-row k-tile -> PSUM strip
    [128, 1024]; exp on ScalarE (no max subtraction: |scores| <~ 2.5,
    mathematically exact softmax) -> fp16 SBUF strip.
  - ctx^T accumulation: lhsT = [V_h | ones] [128, 65], rhs = exp strip;
    row 64 of the accumulator is then the softmax denominator for free.
  - normalize: reciprocal of denom row, partition-broadcast, multiply,
    add bV (exact: sum(attn)=1), write into cat^T tiles.
  - out^T = WO @ cat^T (+ bO) -> DRAM [E, 1024] f32.
"""

import sys

import numpy as np

try:
    import concourse.bass as bass  # noqa: F401
except ImportError:  # fresh grading dir: fall back to the repo path
    sys.path.insert(0, "/opt/trn_rl_repo")

B, S, E, H, DH = 4, 2048, 512, 8, 64
NQ = S // 2  # 1024 query rows per core
ET = E // 128  # 4 E-tiles
KTN = S // 128  # 16 key tiles
NCORES = 8

_CACHE: dict = {}


def _build_nc(debug_taps=False, loop_n=0):
    from contextlib import nullcontext

    import concourse.tile as tile
    from concourse import bacc, mybir

    f16 = mybir.dt.float16
    f32 = mybir.dt.float32
    AF = mybir.ActivationFunctionType

    nc = bacc.Bacc(
        "TRN2", target_bir_lowering=False, debug=False, num_devices=NCORES
    )

    # DRAM parameters (per-core shards; host-packed layouts)
    xq = nc.dram_tensor("xq", [128, ET, NQ], f16, kind="ExternalInput").ap()
    xk = nc.dram_tensor("xk", [128, ET, S], f16, kind="ExternalInput").ap()
    xv = nc.dram_tensor("xv", [128, ET, S], f16, kind="ExternalInput").ap()
    wq = nc.dram_tensor("wq", [128, ET, 4, 128], f16, kind="ExternalInput").ap()
    wk = nc.dram_tensor("wk", [128, ET, 4, 128], f16, kind="ExternalInput").ap()
    wv = nc.dram_tensor("wv", [128, ET, 512], f16, kind="ExternalInput").ap()
    wo = nc.dram_tensor("wo", [128, ET, 512], f16, kind="ExternalInput").ap()
    bq = nc.dram_tensor("bq", [128, 4], f32, kind="ExternalInput").ap()
    bk = nc.dram_tensor("bk", [128, 4], f32, kind="ExternalInput").ap()
    bv = nc.dram_tensor("bv", [64, H], f32, kind="ExternalInput").ap()
    bo = nc.dram_tensor("bo", [128, ET], f32, kind="ExternalInput").ap()
    outT = nc.dram_tensor("outT", [E, NQ], f32, kind="ExternalOutput").ap()
    taps = {}
    if debug_taps:
        taps["dq0"] = nc.dram_tensor("dq0", [64, NQ], f16, kind="ExternalOutput").ap()
        taps["dk0"] = nc.dram_tensor("dk0", [64, S], f16, kind="ExternalOutput").ap()
        taps["dv0"] = nc.dram_tensor("dv0", [128, H * 65], f16, kind="ExternalOutput").ap()
        taps["dps"] = nc.dram_tensor("dps", [128, NQ], f32, kind="ExternalOutput").ap()
        taps["des"] = nc.dram_tensor("des", [128, NQ], f16, kind="ExternalOutput").ap()
        taps["dcx"] = nc.dram_tensor("dcx", [65, NQ], f32, kind="ExternalOutput").ap()
        for et in range(ET):
            taps[f"dcat{et}"] = nc.dram_tensor(
                f"dcat{et}", [128, NQ], f16, kind="ExternalOutput"
            ).ap()
        taps["drecip"] = nc.dram_tensor("drecip", [1, NQ], f32, kind="ExternalOutput").ap()
        taps["drrep"] = nc.dram_tensor("drrep", [64, NQ], f32, kind="ExternalOutput").ap()
        taps["dtmp"] = nc.dram_tensor("dtmp", [64, NQ], f32, kind="ExternalOutput").ap()

    with tile.TileContext(nc) as tc:
        loop_cm = tc.For_i(0, loop_n, 1) if loop_n > 1 else nullcontext()
        with loop_cm, tc.tile_pool(name="const", bufs=1) as const, tc.tile_pool(
            name="work", bufs=1
        ) as work, tc.tile_pool(name="expp", bufs=4) as expp, tc.tile_pool(
            name="spsum", bufs=2, space="PSUM"
        ) as spsum, tc.tile_pool(name="cpsum", bufs=2, space="PSUM") as cpsum:
            # ---- input DMA (ordered so the Q/K projections can start early)
            wq_sb = const.tile([128, ET, 4, 128], f16, name="wq_sb", tag="wq")
            nc.sync.dma_start(wq_sb, wq)
            bq_sb = const.tile([128, 4], f32, name="bq_sb", tag="bq")
            nc.sync.dma_start(bq_sb, bq)
            xq_sb = const.tile([128, ET, NQ], f16, name="xq_sb", tag="xq")
            nc.sync.dma_start(xq_sb, xq)
            wk_sb = const.tile([128, ET, 4, 128], f16, name="wk_sb", tag="wk")
            nc.sync.dma_start(wk_sb, wk)
            bk_sb = const.tile([128, 4], f32, name="bk_sb", tag="bk")
            nc.sync.dma_start(bk_sb, bk)
            xk_sb = const.tile([128, ET, S], f16, name="xk_sb", tag="xk")
            nc.sync.dma_start(xk_sb, xk)
            wv_sb = const.tile([128, ET, 512], f16, name="wv_sb", tag="wv")
            nc.sync.dma_start(wv_sb, wv)
            xv_sb = const.tile([128, ET, S], f16, name="xv_sb", tag="xv")
            nc.sync.dma_start(xv_sb, xv)
            wo_sb = const.tile([128, ET, 512], f16, name="wo_sb", tag="wo")
            nc.sync.dma_start(wo_sb, wo)
            bv_sb = const.tile([64, H], f32, name="bv_sb", tag="bv")
            nc.sync.dma_start(bv_sb, bv)
            bo_sb = const.tile([128, ET], f32, name="bo_sb", tag="bo")
            nc.sync.dma_start(bo_sb, bo)

            # ---- persistent on-chip tensors
            qt = [
                work.tile([64, NQ], f16, name=f"qt{h}", tag=f"qt{h}")
                for h in range(H)
            ]
            kt = [
                work.tile([64, S], f16, name=f"kt{h}", tag=f"kt{h}")
                for h in range(H)
            ]
            vsb = [
                work.tile([128, H, 65], f16, name=f"v{st}", tag=f"v{st}")
                for st in range(KTN)
            ]
            catT = [
                work.tile([128, NQ], f16, name=f"cat{et}", tag=f"cat{et}")
                for et in range(ET)
            ]

            def emit_qt_pair(p):
                ps = spsum.tile([128, NQ], f32, name=f"qtp{p}", tag="strip")
                for qc in range(2):
                    for et in range(ET):
                        nc.tensor.matmul(
                            ps[:, qc * 512 : (qc + 1) * 512],
                            wq_sb[:, et, p, :],
                            xq_sb[:, et, qc * 512 : (qc + 1) * 512],
                            start=(et == 0),
                            stop=(et == ET - 1),
                        )
                nc.vector.tensor_scalar_add(
                    qt[2 * p], ps[0:64, :], bq_sb[0:64, p : p + 1]
                )
                nc.vector.tensor_scalar_add(
                    qt[2 * p + 1], ps[64:128, :], bq_sb[64:128, p : p + 1]
                )

            def emit_kt_pair(p, kc):
                ps = spsum.tile([128, NQ], f32, name=f"ktp{p}_{kc}", tag="strip")
                for n in range(2):
                    for et in range(ET):
                        nc.tensor.matmul(
                            ps[:, n * 512 : (n + 1) * 512],
                            wk_sb[:, et, p, :],
                            xk_sb[
                                :,
                                et,
                                kc * NQ + n * 512 : kc * NQ + (n + 1) * 512,
                            ],
                            start=(et == 0),
                            stop=(et == ET - 1),
                        )
                nc.vector.tensor_scalar_add(
                    kt[2 * p][:, kc * NQ : (kc + 1) * NQ],
                    ps[0:64, :],
                    bk_sb[0:64, p : p + 1],
                )
                nc.vector.tensor_scalar_add(
                    kt[2 * p + 1][:, kc * NQ : (kc + 1) * NQ],
                    ps[64:128, :],
                    bk_sb[64:128, p : p + 1],
                )

            def emit_v(st):
                ps = spsum.tile([128, 512], f32, name=f"vps{st}", tag="strip")
                for et in range(ET):
                    nc.tensor.matmul(
                        ps,
                        xv_sb[:, et, st * 128 : (st + 1) * 128],
                        wv_sb[:, et, :],
                        start=(et == 0),
                        stop=(et == ET - 1),
                    )
                nc.vector.tensor_copy(
                    vsb[st][:, :, 0:64], ps.rearrange("k (h d) -> k h d", h=H)
                )
                nc.vector.memset(vsb[st][:, :, 64:65], 1.0)

            def emit_head(h, extras):
                cx = cpsum.tile([65, NQ], f32, name=f"cx{h}", tag="ctx")
                for st in range(KTN):
                    if extras:
                        extras.pop(0)()
                    ps = spsum.tile(
                        [128, NQ], f32, name=f"sc{h}_{st}", tag="strip"
                    )
                    for qc in range(2):
                        nc.tensor.matmul(
                            ps[:, qc * 512 : (qc + 1) * 512],
                            kt[h][:, st * 128 : (st + 1) * 128],
                            qt[h][:, qc * 512 : (qc + 1) * 512],
                            start=True,
                            stop=True,
                        )
                    es = expp.tile([128, NQ], f16, name=f"es{h}_{st}", tag="exp")
                    nc.scalar.activation(es, ps, AF.Exp)
                    if taps and h == 0 and st == 0:
                        dpss = work.tile([128, NQ], f32, name="dpss", tag="dpss")
                        nc.vector.tensor_copy(dpss, ps)
                        nc.sync.dma_start(taps["dps"], dpss)
                        nc.sync.dma_start(taps["des"], es)
                    for qc in range(2):
                        nc.tensor.matmul(
                            cx[:, qc * 512 : (qc + 1) * 512],
                            vsb[st][:, h, :],
                            es[:, qc * 512 : (qc + 1) * 512],
                            start=(st == 0),
                            stop=(st == KTN - 1),
                        )
                # normalize + bV -> catT
                if taps and h == 0:
                    dcxs = work.tile([65, NQ], f32, name="dcxs", tag="dcxs")
                    nc.vector.tensor_copy(dcxs, cx)
                    nc.sync.dma_start(taps["dcx"], dcxs)
                # NB: reciprocal_approx_fast (custom DVE) drops PSUM partition
                # offsets on HW, and plain reciprocal traps to a slow software
                # handler — so copy the denominator row to SBUF partition 0
                # first, then run the fast approx there.
                srow = work.tile([1, NQ], f32, name=f"sr{h}", tag="srow", bufs=2)
                nc.vector.tensor_copy(srow, cx[64:65, :])
                recip = work.tile([1, NQ], f32, name=f"rc{h}", tag="recip", bufs=2)
                nc.vector.reciprocal_approx_fast(recip, srow)
                rrep = work.tile([64, NQ], f32, name=f"rr{h}", tag="rrep", bufs=2)
                nc.gpsimd.partition_broadcast(rrep, recip)
                tmp = work.tile([64, NQ], f32, name=f"tm{h}", tag="ctmp", bufs=2)
                nc.vector.tensor_mul(tmp, cx[0:64, :], rrep)
                if taps and h == 0:
                    nc.sync.dma_start(taps["drecip"], recip)
                    nc.sync.dma_start(taps["drrep"], rrep)
                    nc.sync.dma_start(taps["dtmp"], tmp)
                po = (h % 2) * 64
                nc.vector.tensor_scalar_add(
                    catT[h // 2][po : po + 64, :], tmp, bv_sb[:, h : h + 1]
                )

            # ---- emission schedule
            emit_qt_pair(0)
            emit_kt_pair(0, 0)
            emit_kt_pair(0, 1)
            emit_v(0)
            emit_v(1)

            extras_by_head: dict = {
                0: [(lambda st=st: emit_v(st)) for st in range(2, KTN)],
                1: [
                    lambda: emit_qt_pair(1),
                    lambda: emit_kt_pair(1, 0),
                    lambda: emit_kt_pair(1, 1),
                ],
                3: [
                    lambda: emit_qt_pair(2),
                    lambda: emit_kt_pair(2, 0),
                    lambda: emit_kt_pair(2, 1),
                ],
                5: [
                    lambda: emit_qt_pair(3),
                    lambda: emit_kt_pair(3, 0),
                    lambda: emit_kt_pair(3, 1),
                ],
            }
            for h in range(H):
                emit_head(h, extras_by_head.get(h, []))

            if taps:
                nc.sync.dma_start(taps["dq0"], qt[0])
                nc.sync.dma_start(taps["dk0"], kt[0])
                nc.sync.dma_start(
                    taps["dv0"], vsb[0].rearrange("k h d -> k (h d)")
                )
                for et in range(ET):
                    nc.sync.dma_start(taps[f"dcat{et}"], catT[et])

            # ---- output projection: out^T = WO @ cat^T + bO
            for eo in range(ET):
                ps = spsum.tile([128, NQ], f32, name=f"wops{eo}", tag="strip")
                for qc in range(2):
                    for et in range(ET):
                        nc.tensor.matmul(
                            ps[:, qc * 512 : (qc + 1) * 512],
                            wo_sb[:, et, eo * 128 : (eo + 1) * 128],
                            catT[et][:, qc * 512 : (qc + 1) * 512],
                            start=(et == 0),
                            stop=(et == ET - 1),
                        )
                osb = work.tile([128, NQ], f32, name=f"osb{eo}", tag="osb", bufs=2)
                nc.vector.tensor_scalar_add(osb, ps, bo_sb[:, eo : eo + 1])
                nc.sync.dma_start(outT[eo * 128 : (eo + 1) * 128, :], osb)

    nc.compile()
    return nc


def _pack_weights(WQ, bQ, WK, bK, WV, bV, WO, bO):
    """Host-side weight packing (layouts documented in _build_nc)."""
    scale = 1.0 / np.sqrt(DH)
    WQs = (np.asarray(WQ, np.float32) * scale).astype(np.float16)
    bQs = np.asarray(bQ, np.float32) * scale
    WKh = np.asarray(WK, np.float32).astype(np.float16)
    WVh = np.asarray(WV, np.float32).astype(np.float16)
    WOh = np.asarray(WO, np.float32).astype(np.float16)

    wq_p = np.zeros((128, ET, 4, 128), np.float16)
    wk_p = np.zeros((128, ET, 4, 128), np.float16)
    for p in range(4):
        for et in range(ET):
            sl = slice(et * 128, (et + 1) * 128)
            wq_p[:, et, p, 0:64] = WQs[2 * p, sl, :]
            wq_p[:, et, p, 64:128] = WQs[2 * p + 1, sl, :]
            wk_p[:, et, p, 0:64] = WKh[2 * p, sl, :]
            wk_p[:, et, p, 64:128] = WKh[2 * p + 1, sl, :]

    wv_p = np.zeros((128, ET, 512), np.float16)
    wo_p = np.zeros((128, ET, 512), np.float16)
    for et in range(ET):
        sl = slice(et * 128, (et + 1) * 128)
        for h in range(H):
            wv_p[:, et, 64 * h : 64 * h + 64] = WVh[h, sl, :]
        wo_p[:, et, :] = WOh[:, sl].T

    bq_p = np.zeros((128, 4), np.float32)
    bk_p = np.zeros((128, 4), np.float32)
    for p in range(4):
        bq_p[0:64, p] = bQs[2 * p]
        bq_p[64:128, p] = bQs[2 * p + 1]
        bk_p[0:64, p] = np.asarray(bK, np.float32)[2 * p]
        bk_p[64:128, p] = np.asarray(bK, np.float32)[2 * p + 1]

    bv_p = np.asarray(bV, np.float32).T.copy()  # [64, H]
    bo_p = np.asarray(bO, np.float32).reshape(ET, 128).T.copy()  # [128, ET]

    return dict(
        wq=wq_p, wk=wk_p, wv=wv_p, wo=wo_p, bq=bq_p, bk=bk_p, bv=bv_p, bo=bo_p
    )


def _pack_seq_T(x2d):
    """[seq, E] f32 -> [128, ET, seq] f16 (transposed, E-tiled)."""
    seq = x2d.shape[0]
    return (
        np.ascontiguousarray(x2d.T)
        .reshape(ET, 128, seq)
        .transpose(1, 0, 2)
        .astype(np.float16)
    )


def prepare(embeddings_q, embeddings_k, embeddings_v, WQ, bQ, WK, bK, WV, bV, WO, bO):
    """Build per-core input maps. Returns in_maps list of dicts."""
    w = _pack_weights(WQ, bQ, WK, bK, WV, bV, WO, bO)
    eq = np.asarray(embeddings_q, np.float32)
    ek = np.asarray(embeddings_k, np.float32)
    ev = np.asarray(embeddings_v, np.float32)
    in_maps = []
    for b in range(B):
        xk_p = _pack_seq_T(ek[b])
        xv_p = _pack_seq_T(ev[b])
        for half in range(2):
            xq_p = _pack_seq_T(eq[b, half * NQ : (half + 1) * NQ, :])
            m = {"xq": xq_p, "xk": xk_p, "xv": xv_p}
            m.update(w)
            in_maps.append(m)
    return in_maps


def get_nc():
    if "nc" not in _CACHE:
        _CACHE["nc"] = _build_nc()
    return _CACHE["nc"]


def assemble(core_outs):
    """core_outs: list of 8 dicts with 'outT' [E, NQ] f32 -> [B, S, E]."""
    out = np.empty((B, S, E), np.float32)
    for b in range(B):
        for half in range(2):
            c = 2 * b + half
            out[b, half * NQ : (half + 1) * NQ, :] = core_outs[c]["outT"].T
    return out


def kernel(**inputs):
    from concourse.bass_utils import run_bass_kernel_spmd

    nc = get_nc()
    in_maps = prepare(**inputs)
    res = run_bass_kernel_spmd(nc, in_maps, core_ids=list(range(NCORES)))
    return assemble(res.results)


# revision 17
# speedup vs baseline: 6.4234x; 1.2243x over previous
"""Trainium2 Bass kernel: multi-head encoder-decoder attention.

Problem: B=4, S=2048, E=512, H=8, DH=64 (f32 reference).

Sharding: (batch, query-half) -> 8 cores. Core c = 2*b + half computes
all 8 heads of attention for query rows [half*1024, (half+1)*1024) of
batch b against the batch's full K/V, plus the output projection for
those rows. No cross-core communication; the host concatenates the
eight [E, 1024] outputs (transposed) into [B, S, E].

Per-core layout strategy (all activations fp16, PSUM accumulation f32):
  - Host pre-transposes embeddings to [E, seq] so projections need no
    on-chip transposes: Q^T/K^T per head are produced directly as
    [DH, seq] (scale 1/sqrt(DH) and bQ folded into WQ/bQ host-side).
  - scores^T[k, q] = K'^T.T @ Q'^T per 128-row k-tile -> PSUM strip
    [128, 1024]; exp on ScalarE (no max subtraction: |scores| <~ 2.5,
    mathematically exact softmax) -> fp16 SBUF strip.
  - ctx^T accumulation: lhsT = [V_h | ones] [128, 65], rhs = exp strip;
    row 64 of the accumulator is then the softmax denominator for free.
  - normalize: reciprocal of denom row, partition-broadcast, multiply,
    add bV (exact: sum(attn)=1), write into cat^T tiles.
  - out^T = WO @ cat^T (+ bO) -> DRAM [E, 1024] f32.
"""

import sys

import numpy as np

try:
    import concourse.bass as bass  # noqa: F401
except ImportError:  # fresh grading dir: fall back to the repo path
    sys.path.insert(0, "/opt/trn_rl_repo")

B, S, E, H, DH = 4, 2048, 512, 8, 64
NQ = S // 2  # 1024 query rows per core
ET = E // 128  # 4 E-tiles
KTN = S // 128  # 16 key tiles
NCORES = 8

_CACHE: dict = {}


def _build_nc(debug_taps=False, loop_n=0):
    from contextlib import nullcontext

    import concourse.tile as tile
    from concourse import bacc, mybir

    f16 = mybir.dt.float16
    f32 = mybir.dt.float32
    AF = mybir.ActivationFunctionType

    nc = bacc.Bacc(
        "TRN2", target_bir_lowering=False, debug=False, num_devices=NCORES
    )

    # DRAM parameters (per-core shards; host-packed layouts)
    xq = nc.dram_tensor("xq", [128, ET, NQ], f16, kind="ExternalInput").ap()
    xk = nc.dram_tensor("xk", [128, ET, S], f16, kind="ExternalInput").ap()
    xv = nc.dram_tensor("xv", [128, ET, S], f16, kind="ExternalInput").ap()
    wq = nc.dram_tensor("wq", [128, ET, 4, 128], f16, kind="ExternalInput").ap()
    wk = nc.dram_tensor("wk", [128, ET, 4, 128], f16, kind="ExternalInput").ap()
    wv = nc.dram_tensor("wv", [128, ET, 512], f16, kind="ExternalInput").ap()
    wo = nc.dram_tensor("wo", [128, ET, 512], f16, kind="ExternalInput").ap()
    bq = nc.dram_tensor("bq", [128, 4], f32, kind="ExternalInput").ap()
    bk = nc.dram_tensor("bk", [128, 4], f32, kind="ExternalInput").ap()
    bv = nc.dram_tensor("bv", [64, H], f32, kind="ExternalInput").ap()
    bo = nc.dram_tensor("bo", [128, ET], f32, kind="ExternalInput").ap()
    outT = nc.dram_tensor("outT", [E, NQ], f32, kind="ExternalOutput").ap()
    taps = {}
    if debug_taps:
        taps["dq0"] = nc.dram_tensor("dq0", [64, NQ], f16, kind="ExternalOutput").ap()
        taps["dk0"] = nc.dram_tensor("dk0", [64, S], f16, kind="ExternalOutput").ap()
        taps["dv0"] = nc.dram_tensor("dv0", [128, H * 65], f16, kind="ExternalOutput").ap()
        taps["dps"] = nc.dram_tensor("dps", [128, NQ], f32, kind="ExternalOutput").ap()
        taps["des"] = nc.dram_tensor("des", [128, NQ], f16, kind="ExternalOutput").ap()
        taps["dcx"] = nc.dram_tensor("dcx", [65, NQ], f32, kind="ExternalOutput").ap()
        for et in range(ET):
            taps[f"dcat{et}"] = nc.dram_tensor(
                f"dcat{et}", [128, NQ], f16, kind="ExternalOutput"
            ).ap()
        taps["drecip"] = nc.dram_tensor("drecip", [1, NQ], f32, kind="ExternalOutput").ap()
        taps["drrep"] = nc.dram_tensor("drrep", [64, NQ], f32, kind="ExternalOutput").ap()
        taps["dtmp"] = nc.dram_tensor("dtmp", [64, NQ], f32, kind="ExternalOutput").ap()

    with tile.TileContext(nc) as tc:
        loop_cm = tc.For_i(0, loop_n, 1) if loop_n > 1 else nullcontext()
        with loop_cm, tc.tile_pool(name="const", bufs=1) as const, tc.tile_pool(
            name="work", bufs=1
        ) as work, tc.tile_pool(name="expp", bufs=4) as expp, tc.tile_pool(
            name="spsum", bufs=2, space="PSUM"
        ) as spsum, tc.tile_pool(name="cpsum", bufs=2, space="PSUM") as cpsum:
            # ---- input DMA (ordered so the Q/K projections can start early)
            wq_sb = const.tile([128, ET, 4, 128], f16, name="wq_sb", tag="wq")
            nc.sync.dma_start(wq_sb, wq)
            bq_sb = const.tile([128, 4], f32, name="bq_sb", tag="bq")
            nc.sync.dma_start(bq_sb, bq)
            xq_sb = const.tile([128, ET, NQ], f16, name="xq_sb", tag="xq")
            nc.sync.dma_start(xq_sb, xq)
            wk_sb = const.tile([128, ET, 4, 128], f16, name="wk_sb", tag="wk")
            nc.sync.dma_start(wk_sb, wk)
            bk_sb = const.tile([128, 4], f32, name="bk_sb", tag="bk")
            nc.sync.dma_start(bk_sb, bk)
            xk_sb = const.tile([128, ET, S], f16, name="xk_sb", tag="xk")
            nc.sync.dma_start(xk_sb, xk)
            wv_sb = const.tile([128, ET, 512], f16, name="wv_sb", tag="wv")
            nc.sync.dma_start(wv_sb, wv)
            xv_sb = const.tile([128, ET, S], f16, name="xv_sb", tag="xv")
            nc.sync.dma_start(xv_sb, xv)
            wo_sb = const.tile([128, ET, 512], f16, name="wo_sb", tag="wo")
            nc.sync.dma_start(wo_sb, wo)
            bv_sb = const.tile([64, H], f32, name="bv_sb", tag="bv")
            nc.sync.dma_start(bv_sb, bv)
            bo_sb = const.tile([128, ET], f32, name="bo_sb", tag="bo")
            nc.sync.dma_start(bo_sb, bo)

            # ---- persistent on-chip tensors
            qt = [
                work.tile([64, NQ], f16, name=f"qt{h}", tag=f"qt{h}")
                for h in range(H)
            ]
            kt = [
                work.tile([64, S], f16, name=f"kt{h}", tag=f"kt{h}")
                for h in range(H)
            ]
            vsb = [
                work.tile([128, H, 65], f16, name=f"v{st}", tag=f"v{st}")
                for st in range(KTN)
            ]
            catT = [
                work.tile([128, NQ], f16, name=f"cat{et}", tag=f"cat{et}")
                for et in range(ET)
            ]

            def emit_qt_pair(p):
                ps = spsum.tile([128, NQ], f32, name=f"qtp{p}", tag="strip")
                for qc in range(2):
                    for et in range(ET):
                        nc.tensor.matmul(
                            ps[:, qc * 512 : (qc + 1) * 512],
                            wq_sb[:, et, p, :],
                            xq_sb[:, et, qc * 512 : (qc + 1) * 512],
                            start=(et == 0),
                            stop=(et == ET - 1),
                        )
                nc.vector.tensor_scalar_add(
                    qt[2 * p], ps[0:64, :], bq_sb[0:64, p : p + 1]
                )
                nc.vector.tensor_scalar_add(
                    qt[2 * p + 1], ps[64:128, :], bq_sb[64:128, p : p + 1]
                )

            def emit_kt_pair(p, kc):
                ps = spsum.tile([128, NQ], f32, name=f"ktp{p}_{kc}", tag="strip")
                for n in range(2):
                    for et in range(ET):
                        nc.tensor.matmul(
                            ps[:, n * 512 : (n + 1) * 512],
                            wk_sb[:, et, p, :],
                            xk_sb[
                                :,
                                et,
                                kc * NQ + n * 512 : kc * NQ + (n + 1) * 512,
                            ],
                            start=(et == 0),
                            stop=(et == ET - 1),
                        )
                nc.vector.tensor_scalar_add(
                    kt[2 * p][:, kc * NQ : (kc + 1) * NQ],
                    ps[0:64, :],
                    bk_sb[0:64, p : p + 1],
                )
                nc.vector.tensor_scalar_add(
                    kt[2 * p + 1][:, kc * NQ : (kc + 1) * NQ],
                    ps[64:128, :],
                    bk_sb[64:128, p : p + 1],
                )

            def emit_v(st):
                ps = spsum.tile([128, 512], f32, name=f"vps{st}", tag="strip")
                for et in range(ET):
                    nc.tensor.matmul(
                        ps,
                        xv_sb[:, et, st * 128 : (st + 1) * 128],
                        wv_sb[:, et, :],
                        start=(et == 0),
                        stop=(et == ET - 1),
                    )
                nc.vector.tensor_copy(
                    vsb[st][:, :, 0:64], ps.rearrange("k (h d) -> k h d", h=H)
                )
                nc.vector.memset(vsb[st][:, :, 64:65], 1.0)

            def emit_head(h, extras):
                cx = cpsum.tile([65, NQ], f32, name=f"cx{h}", tag="ctx")
                for st in range(KTN):
                    if extras:
                        extras.pop(0)()
                    ps = spsum.tile(
                        [128, NQ], f32, name=f"sc{h}_{st}", tag="strip"
                    )
                    for qc in range(2):
                        nc.tensor.matmul(
                            ps[:, qc * 512 : (qc + 1) * 512],
                            kt[h][:, st * 128 : (st + 1) * 128],
                            qt[h][:, qc * 512 : (qc + 1) * 512],
                            start=True,
                            stop=True,
                        )
                    es = expp.tile([128, NQ], f16, name=f"es{h}_{st}", tag="exp")
                    nc.scalar.activation(es, ps, AF.Exp)
                    if taps and h == 0 and st == 0:
                        dpss = work.tile([128, NQ], f32, name="dpss", tag="dpss")
                        nc.vector.tensor_copy(dpss, ps)
                        nc.sync.dma_start(taps["dps"], dpss)
                        nc.sync.dma_start(taps["des"], es)
                    for qc in range(2):
                        nc.tensor.matmul(
                            cx[:, qc * 512 : (qc + 1) * 512],
                            vsb[st][:, h, :],
                            es[:, qc * 512 : (qc + 1) * 512],
                            start=(st == 0),
                            stop=(st == KTN - 1),
                        )
                # normalize + bV -> catT
                if taps and h == 0:
                    dcxs = work.tile([65, NQ], f32, name="dcxs", tag="dcxs")
                    nc.vector.tensor_copy(dcxs, cx)
                    nc.sync.dma_start(taps["dcx"], dcxs)
                # NB: reciprocal_approx_fast (custom DVE) drops PSUM partition
                # offsets on HW, and plain reciprocal traps to a slow software
                # handler — so copy the denominator row to SBUF partition 0
                # first, then run the fast approx there.
                srow = work.tile([1, NQ], f32, name=f"sr{h}", tag="srow", bufs=2)
                nc.vector.tensor_copy(srow, cx[64:65, :])
                recip = work.tile([1, NQ], f32, name=f"rc{h}", tag="recip", bufs=2)
                nc.vector.reciprocal_approx_fast(recip, srow)
                rrep = work.tile([64, NQ], f32, name=f"rr{h}", tag="rrep", bufs=2)
                nc.gpsimd.partition_broadcast(rrep, recip)
                tmp = work.tile([64, NQ], f32, name=f"tm{h}", tag="ctmp", bufs=2)
                nc.vector.tensor_mul(tmp, cx[0:64, :], rrep)
                if taps and h == 0:
                    nc.sync.dma_start(taps["drecip"], recip)
                    nc.sync.dma_start(taps["drrep"], rrep)
                    nc.sync.dma_start(taps["dtmp"], tmp)
                po = (h % 2) * 64
                nc.vector.tensor_scalar_add(
                    catT[h // 2][po : po + 64, :], tmp, bv_sb[:, h : h + 1]
                )

            # ---- emission schedule: all projections first (PE-only phase
            # while ACT ramps), then a clean ACT-paced attention loop.
            for p in range(4):
                emit_qt_pair(p)
            for p in range(4):
                for kc in range(2):
                    emit_kt_pair(p, kc)
            for st in range(KTN):
                emit_v(st)
            for h in range(H):
                emit_head(h, [])

            if taps:
                nc.sync.dma_start(taps["dq0"], qt[0])
                nc.sync.dma_start(taps["dk0"], kt[0])
                nc.sync.dma_start(
                    taps["dv0"], vsb[0].rearrange("k h d -> k (h d)")
                )
                for et in range(ET):
                    nc.sync.dma_start(taps[f"dcat{et}"], catT[et])

            # ---- output projection: out^T = WO @ cat^T + bO
            for eo in range(ET):
                ps = spsum.tile([128, NQ], f32, name=f"wops{eo}", tag="strip")
                for qc in range(2):
                    for et in range(ET):
                        nc.tensor.matmul(
                            ps[:, qc * 512 : (qc + 1) * 512],
                            wo_sb[:, et, eo * 128 : (eo + 1) * 128],
                            catT[et][:, qc * 512 : (qc + 1) * 512],
                            start=(et == 0),
                            stop=(et == ET - 1),
                        )
                osb = work.tile([128, NQ], f32, name=f"osb{eo}", tag="osb", bufs=2)
                nc.vector.tensor_scalar_add(osb, ps, bo_sb[:, eo : eo + 1])
                nc.sync.dma_start(outT[eo * 128 : (eo + 1) * 128, :], osb)

    nc.compile()
    return nc


def _pack_weights(WQ, bQ, WK, bK, WV, bV, WO, bO):
    """Host-side weight packing (layouts documented in _build_nc)."""
    scale = 1.0 / np.sqrt(DH)
    WQs = (np.asarray(WQ, np.float32) * scale).astype(np.float16)
    bQs = np.asarray(bQ, np.float32) * scale
    WKh = np.asarray(WK, np.float32).astype(np.float16)
    WVh = np.asarray(WV, np.float32).astype(np.float16)
    WOh = np.asarray(WO, np.float32).astype(np.float16)

    wq_p = np.zeros((128, ET, 4, 128), np.float16)
    wk_p = np.zeros((128, ET, 4, 128), np.float16)
    for p in range(4):
        for et in range(ET):
            sl = slice(et * 128, (et + 1) * 128)
            wq_p[:, et, p, 0:64] = WQs[2 * p, sl, :]
            wq_p[:, et, p, 64:128] = WQs[2 * p + 1, sl, :]
            wk_p[:, et, p, 0:64] = WKh[2 * p, sl, :]
            wk_p[:, et, p, 64:128] = WKh[2 * p + 1, sl, :]

    wv_p = np.zeros((128, ET, 512), np.float16)
    wo_p = np.zeros((128, ET, 512), np.float16)
    for et in range(ET):
        sl = slice(et * 128, (et + 1) * 128)
        for h in range(H):
            wv_p[:, et, 64 * h : 64 * h + 64] = WVh[h, sl, :]
        wo_p[:, et, :] = WOh[:, sl].T

    bq_p = np.zeros((128, 4), np.float32)
    bk_p = np.zeros((128, 4), np.float32)
    for p in range(4):
        bq_p[0:64, p] = bQs[2 * p]
        bq_p[64:128, p] = bQs[2 * p + 1]
        bk_p[0:64, p] = np.asarray(bK, np.float32)[2 * p]
        bk_p[64:128, p] = np.asarray(bK, np.float32)[2 * p + 1]

    bv_p = np.asarray(bV, np.float32).T.copy()  # [64, H]
    bo_p = np.asarray(bO, np.float32).reshape(ET, 128).T.copy()  # [128, ET]

    return dict(
        wq=wq_p, wk=wk_p, wv=wv_p, wo=wo_p, bq=bq_p, bk=bk_p, bv=bv_p, bo=bo_p
    )


def _pack_seq_T(x2d):
    """[seq, E] f32 -> [128, ET, seq] f16 (transposed, E-tiled)."""
    seq = x2d.shape[0]
    return (
        np.ascontiguousarray(x2d.T)
        .reshape(ET, 128, seq)
        .transpose(1, 0, 2)
        .astype(np.float16)
    )


def prepare(embeddings_q, embeddings_k, embeddings_v, WQ, bQ, WK, bK, WV, bV, WO, bO):
    """Build per-core input maps. Returns in_maps list of dicts."""
    w = _pack_weights(WQ, bQ, WK, bK, WV, bV, WO, bO)
    eq = np.asarray(embeddings_q, np.float32)
    ek = np.asarray(embeddings_k, np.float32)
    ev = np.asarray(embeddings_v, np.float32)
    in_maps = []
    for b in range(B):
        xk_p = _pack_seq_T(ek[b])
        xv_p = _pack_seq_T(ev[b])
        for half in range(2):
            xq_p = _pack_seq_T(eq[b, half * NQ : (half + 1) * NQ, :])
            m = {"xq": xq_p, "xk": xk_p, "xv": xv_p}
            m.update(w)
            in_maps.append(m)
    return in_maps


def get_nc():
    if "nc" not in _CACHE:
        _CACHE["nc"] = _build_nc()
    return _CACHE["nc"]


def assemble(core_outs):
    """core_outs: list of 8 dicts with 'outT' [E, NQ] f32 -> [B, S, E]."""
    out = np.empty((B, S, E), np.float32)
    for b in range(B):
        for half in range(2):
            c = 2 * b + half
            out[b, half * NQ : (half + 1) * NQ, :] = core_outs[c]["outT"].T
    return out


def kernel(**inputs):
    from concourse.bass_utils import run_bass_kernel_spmd

    nc = get_nc()
    in_maps = prepare(**inputs)
    res = run_bass_kernel_spmd(nc, in_maps, core_ids=list(range(NCORES)))
    return assemble(res.results)


# revision 20
# speedup vs baseline: 6.5497x; 1.0196x over previous
"""Trainium2 Bass kernel: multi-head encoder-decoder attention.

Problem: B=4, S=2048, E=512, H=8, DH=64 (f32 reference).

Sharding: (batch, query-half) -> 8 cores. Core c = 2*b + half computes
all 8 heads of attention for query rows [half*1024, (half+1)*1024) of
batch b against the batch's full K/V, plus the output projection for
those rows. No cross-core communication; the host concatenates the
eight [E, 1024] outputs (transposed) into [B, S, E].

Per-core pipeline (activations fp16, PSUM accumulation f32):
  - Host pre-transposes embeddings to [E, seq] so projections need no
    on-chip transposes: Q^T/K^T per head are produced directly as
    [DH, seq] (scale 1/sqrt(DH) and bQ folded into WQ/bQ host-side).
  - scores^T[k, q] = K'^T.T @ Q'^T per 128-row k-tile -> PSUM strip
    [128, 1024]; exp on ScalarE (no max subtraction: |scores| <~ 2.5,
    mathematically exact softmax) -> fp16 SBUF strip. ScalarE is the
    bottleneck engine (~134us of exp); everything else hides under it.
  - ctx^T accumulation: lhsT = [V_h | ones] [128, 65], rhs = exp strip;
    row 64 of the accumulator is the softmax denominator for free.
  - normalize: reciprocal of denom row, partition-broadcast, multiply,
    add bV (exact: sum(attn)=1), write into cat^T tiles.
  - out^T = WO @ cat^T + bO accumulated in SBUF per head-pair so the
    tail after the last head is short. DRAM out [E, 1024] f32.

PSUM budget (8 banks): exp strips 2x2 + ctx 1x2 + aux (proj/WO) 1x2.
Projection bursts use the dedicated aux pool so they never stall the
ScalarE-paced strip pipeline; a 10-deep fp16 exp ring lets ctx/PE lag
behind ScalarE during bursts without stalling it.
"""

import sys

import numpy as np

try:
    import concourse.bass as bass  # noqa: F401
except ImportError:  # fresh grading dir: fall back to the repo path
    sys.path.insert(0, "/opt/trn_rl_repo")

B, S, E, H, DH = 4, 2048, 512, 8, 64
NQ = S // 2  # 1024 query rows per core
ET = E // 128  # 4 E-tiles
KTN = S // 128  # 16 key tiles
NCORES = 8

_CACHE: dict = {}


def _build_nc(debug_taps=False, loop_n=0):
    from contextlib import nullcontext

    import concourse.tile as tile
    from concourse import bacc, mybir

    f16 = mybir.dt.float16
    f32 = mybir.dt.float32
    AF = mybir.ActivationFunctionType

    nc = bacc.Bacc(
        "TRN2", target_bir_lowering=False, debug=False, num_devices=NCORES
    )

    # DRAM parameters (per-core shards; host-packed layouts)
    xq = nc.dram_tensor("xq", [128, ET, NQ], f16, kind="ExternalInput").ap()
    xk = nc.dram_tensor("xk", [128, ET, S], f16, kind="ExternalInput").ap()
    xv = nc.dram_tensor("xv", [128, ET, S], f16, kind="ExternalInput").ap()
    wq = nc.dram_tensor("wq", [128, ET, 4, 128], f16, kind="ExternalInput").ap()
    wk = nc.dram_tensor("wk", [128, ET, 4, 128], f16, kind="ExternalInput").ap()
    wv = nc.dram_tensor("wv", [128, ET, 512], f16, kind="ExternalInput").ap()
    wo = nc.dram_tensor("wo", [128, ET, 512], f16, kind="ExternalInput").ap()
    bq = nc.dram_tensor("bq", [128, 4], f32, kind="ExternalInput").ap()
    bk = nc.dram_tensor("bk", [128, 4], f32, kind="ExternalInput").ap()
    bv = nc.dram_tensor("bv", [64, H], f32, kind="ExternalInput").ap()
    bo = nc.dram_tensor("bo", [128, ET], f32, kind="ExternalInput").ap()
    outT = nc.dram_tensor("outT", [E, NQ], f32, kind="ExternalOutput").ap()
    taps = {}
    if debug_taps:
        for et in range(ET):
            taps[f"dcat{et}"] = nc.dram_tensor(
                f"dcat{et}", [128, NQ], f16, kind="ExternalOutput"
            ).ap()

    with tile.TileContext(nc) as tc:
        loop_cm = tc.For_i(0, loop_n, 1) if loop_n > 1 else nullcontext()
        with loop_cm, tc.tile_pool(name="const", bufs=1) as const, tc.tile_pool(
            name="work", bufs=1
        ) as work, tc.tile_pool(name="expp", bufs=10) as expp, tc.tile_pool(
            name="spsum", bufs=2, space="PSUM"
        ) as spsum, tc.tile_pool(
            name="cpsum", bufs=1, space="PSUM"
        ) as cpsum, tc.tile_pool(
            name="apsum", bufs=1, space="PSUM"
        ) as apsum:
            # ---- input DMA, ordered by first consumption:
            # QT pair0 (wq,bq,xq) -> KT pair0 kc0 (wk,bk,xk half0) -> first V
            # tiles (wv, xv quarter0) -> rest.
            wq_sb = const.tile([128, ET, 4, 128], f16, name="wq_sb", tag="wq")
            nc.sync.dma_start(wq_sb, wq)
            bq_sb = const.tile([128, 4], f32, name="bq_sb", tag="bq")
            nc.sync.dma_start(bq_sb, bq)
            xq_sb = const.tile([128, ET, NQ], f16, name="xq_sb", tag="xq")
            nc.sync.dma_start(xq_sb, xq)
            wk_sb = const.tile([128, ET, 4, 128], f16, name="wk_sb", tag="wk")
            nc.sync.dma_start(wk_sb, wk)
            bk_sb = const.tile([128, 4], f32, name="bk_sb", tag="bk")
            nc.sync.dma_start(bk_sb, bk)
            xk_sb = const.tile([128, ET, S], f16, name="xk_sb", tag="xk")
            nc.sync.dma_start(xk_sb[:, :, 0:NQ], xk[:, :, 0:NQ])
            wv_sb = const.tile([128, ET, 512], f16, name="wv_sb", tag="wv")
            nc.sync.dma_start(wv_sb, wv)
            bv_sb = const.tile([64, H], f32, name="bv_sb", tag="bv")
            nc.sync.dma_start(bv_sb, bv)
            xv_sb = const.tile([128, ET, S], f16, name="xv_sb", tag="xv")
            nc.sync.dma_start(xv_sb[:, :, 0:512], xv[:, :, 0:512])
            nc.sync.dma_start(xk_sb[:, :, NQ:S], xk[:, :, NQ:S])
            for c4 in range(1, 4):
                nc.sync.dma_start(
                    xv_sb[:, :, c4 * 512 : (c4 + 1) * 512],
                    xv[:, :, c4 * 512 : (c4 + 1) * 512],
                )
            wo_sb = const.tile([128, ET, 512], f16, name="wo_sb", tag="wo")
            nc.sync.dma_start(wo_sb, wo)
            bo_sb = const.tile([128, ET], f32, name="bo_sb", tag="bo")
            nc.sync.dma_start(bo_sb, bo)

            # ---- persistent on-chip tensors
            qt = [
                work.tile([64, NQ], f16, name=f"qt{h}", tag=f"qt{h}")
                for h in range(H)
            ]
            kt = [
                work.tile([64, S], f16, name=f"kt{h}", tag=f"kt{h}")
                for h in range(H)
            ]
            vsb = [
                work.tile([128, H, 65], f16, name=f"v{st}", tag=f"v{st}")
                for st in range(KTN)
            ]
            catT = [
                work.tile([128, NQ], f16, name=f"cat{et}", tag=f"cat{et}")
                for et in range(ET)
            ]
            # out^T accumulators (f32, SBUF): one per output e-tile
            acc = [
                work.tile([128, NQ], f32, name=f"acc{eo}", tag=f"acc{eo}")
                for eo in range(ET)
            ]

            def emit_qt_pair(p):
                ps = apsum.tile([128, NQ], f32, name=f"qtp{p}", tag="aux")
                for qc in range(2):
                    for et in range(ET):
                        nc.tensor.matmul(
                            ps[:, qc * 512 : (qc + 1) * 512],
                            wq_sb[:, et, p, :],
                            xq_sb[:, et, qc * 512 : (qc + 1) * 512],
                            start=(et == 0),
                            stop=(et == ET - 1),
                        )
                nc.vector.tensor_scalar_add(
                    qt[2 * p], ps[0:64, :], bq_sb[0:64, p : p + 1]
                )
                nc.vector.tensor_scalar_add(
                    qt[2 * p + 1], ps[64:128, :], bq_sb[64:128, p : p + 1]
                )

            def emit_kt_pair(p, kc):
                ps = apsum.tile([128, NQ], f32, name=f"ktp{p}_{kc}", tag="aux")
                for n in range(2):
                    for et in range(ET):
                        nc.tensor.matmul(
                            ps[:, n * 512 : (n + 1) * 512],
                            wk_sb[:, et, p, :],
                            xk_sb[
                                :,
                                et,
                                kc * NQ + n * 512 : kc * NQ + (n + 1) * 512,
                            ],
                            start=(et == 0),
                            stop=(et == ET - 1),
                        )
                nc.vector.tensor_scalar_add(
                    kt[2 * p][:, kc * NQ : (kc + 1) * NQ],
                    ps[0:64, :],
                    bk_sb[0:64, p : p + 1],
                )
                nc.vector.tensor_scalar_add(
                    kt[2 * p + 1][:, kc * NQ : (kc + 1) * NQ],
                    ps[64:128, :],
                    bk_sb[64:128, p : p + 1],
                )

            def emit_v(st):
                ps = apsum.tile([128, 512], f32, name=f"vps{st}", tag="aux")
                for et in range(ET):
                    nc.tensor.matmul(
                        ps,
                        xv_sb[:, et, st * 128 : (st + 1) * 128],
                        wv_sb[:, et, :],
                        start=(et == 0),
                        stop=(et == ET - 1),
                    )
                nc.vector.tensor_copy(
                    vsb[st][:, :, 0:64], ps.rearrange("k (h d) -> k h d", h=H)
                )
                nc.vector.memset(vsb[st][:, :, 64:65], 1.0)

            def emit_wo_partial(p):
                # out^T partial for contraction e-tile et=p, all 4 eo-tiles;
                # accumulated into SBUF acc (bias folded into the first one).
                for eo in range(ET):
                    ps = apsum.tile([128, NQ], f32, name=f"wop{p}_{eo}", tag="aux")
                    for qc in range(2):
                        nc.tensor.matmul(
                            ps[:, qc * 512 : (qc + 1) * 512],
                            wo_sb[:, p, eo * 128 : (eo + 1) * 128],
                            catT[p][:, qc * 512 : (qc + 1) * 512],
                            start=True,
                            stop=True,
                        )
                    if p == 0:
                        nc.vector.tensor_scalar_add(
                            acc[eo], ps, bo_sb[:, eo : eo + 1]
                        )
                    else:
                        nc.vector.tensor_add(acc[eo], acc[eo], ps)

            # ---- attention: software pipeline. scores/exp run CTX_LAG strips
            # ahead of ctx so PE detours (proj/WO bursts, cx rotation waits)
            # never stall ScalarE: the 10-deep exp ring buffers the lag.
            CTX_LAG = 4
            ctx_queue: list = []
            cx_ref: dict = {}

            def emit_normalize(h):
                # cx is released by two quick copies; the reciprocal chain then
                # runs off the critical path entirely in SBUF.
                # NB: reciprocal_approx_fast (custom DVE) drops PSUM partition
                # offsets on HW, and plain reciprocal traps to a slow software
                # handler - hence the partition-0 staging copies.
                cx = cx_ref.pop(h)
                srow = work.tile([1, NQ], f32, name=f"sr{h}", tag="srow", bufs=2)
                nc.vector.tensor_copy(srow, cx[64:65, :])
                craw = work.tile([64, NQ], f32, name=f"cr{h}", tag="craw", bufs=2)
                nc.vector.tensor_copy(craw, cx[0:64, :])
                recip = work.tile([1, NQ], f32, name=f"rc{h}", tag="recip", bufs=2)
                nc.vector.reciprocal_approx_fast(recip, srow)
                rrep = work.tile([64, NQ], f32, name=f"rr{h}", tag="rrep", bufs=2)
                nc.gpsimd.partition_broadcast(rrep, recip)
                tmp = work.tile([64, NQ], f32, name=f"tm{h}", tag="ctmp", bufs=2)
                nc.vector.tensor_mul(tmp, craw, rrep)
                po = (h % 2) * 64
                nc.vector.tensor_scalar_add(
                    catT[h // 2][po : po + 64, :], tmp, bv_sb[:, h : h + 1]
                )

            def emit_ctx_one():
                h, st, es = ctx_queue.pop(0)
                if st == 0:
                    cx_ref[h] = cpsum.tile([65, NQ], f32, name=f"cx{h}", tag="ctx")
                cx = cx_ref[h]
                for qc in range(2):
                    nc.tensor.matmul(
                        cx[:, qc * 512 : (qc + 1) * 512],
                        vsb[st][:, h, :],
                        es[:, qc * 512 : (qc + 1) * 512],
                        start=(st == 0),
                        stop=(st == KTN - 1),
                    )
                if st == KTN - 1:
                    emit_normalize(h)

            def emit_head(h, extras):
                for st in range(KTN):
                    fn = extras.get(st)
                    if fn is not None:
                        fn()
                    ps = spsum.tile(
                        [128, NQ], f32, name=f"sc{h}_{st}", tag="strip"
                    )
                    for qc in range(2):
                        nc.tensor.matmul(
                            ps[:, qc * 512 : (qc + 1) * 512],
                            kt[h][:, st * 128 : (st + 1) * 128],
                            qt[h][:, qc * 512 : (qc + 1) * 512],
                            start=True,
                            stop=True,
                        )
                    es = expp.tile([128, NQ], f16, name=f"es{h}_{st}", tag="exp")
                    nc.scalar.activation(es, ps, AF.Exp)
                    ctx_queue.append((h, st, es))
                    if len(ctx_queue) > CTX_LAG:
                        emit_ctx_one()

            # ---- emission schedule.
            # Minimal prefix before head 0 (pair 0 kc0 + first V tiles); the
            # remaining projections / WO partials ride inside heads as small
            # aux-pool bursts.
            emit_qt_pair(0)
            emit_kt_pair(0, 0)
            emit_v(0)
            emit_v(1)
            emit_v(2)

            h0_extras = {0: lambda: emit_kt_pair(0, 1)}
            for i, st in enumerate(range(3, KTN)):
                h0_extras[i + 1] = lambda st=st: emit_v(st)
            extras_by_head = {
                0: h0_extras,
                1: {
                    0: lambda: emit_qt_pair(1),
                    1: lambda: emit_kt_pair(1, 0),
                    2: lambda: emit_kt_pair(1, 1),
                },
                2: {10: lambda: emit_wo_partial(0)},
                3: {
                    0: lambda: emit_qt_pair(2),
                    1: lambda: emit_kt_pair(2, 0),
                    2: lambda: emit_kt_pair(2, 1),
                },
                4: {10: lambda: emit_wo_partial(1)},
                5: {
                    0: lambda: emit_qt_pair(3),
                    1: lambda: emit_kt_pair(3, 0),
                    2: lambda: emit_kt_pair(3, 1),
                },
                6: {10: lambda: emit_wo_partial(2)},
            }
            for h in range(H):
                emit_head(h, extras_by_head.get(h, {}))
            while ctx_queue:
                emit_ctx_one()
            emit_wo_partial(3)

            for eo in range(ET):
                nc.sync.dma_start(outT[eo * 128 : (eo + 1) * 128, :], acc[eo])

            if taps:
                for et in range(ET):
                    nc.sync.dma_start(taps[f"dcat{et}"], catT[et])

    nc.compile()
    return nc


def _pack_weights(WQ, bQ, WK, bK, WV, bV, WO, bO):
    """Host-side weight packing (layouts documented in _build_nc)."""
    scale = 1.0 / np.sqrt(DH)
    WQs = (np.asarray(WQ, np.float32) * scale).astype(np.float16)
    bQs = np.asarray(bQ, np.float32) * scale
    WKh = np.asarray(WK, np.float32).astype(np.float16)
    WVh = np.asarray(WV, np.float32).astype(np.float16)
    WOh = np.asarray(WO, np.float32).astype(np.float16)

    wq_p = np.zeros((128, ET, 4, 128), np.float16)
    wk_p = np.zeros((128, ET, 4, 128), np.float16)
    for p in range(4):
        for et in range(ET):
            sl = slice(et * 128, (et + 1) * 128)
            wq_p[:, et, p, 0:64] = WQs[2 * p, sl, :]
            wq_p[:, et, p, 64:128] = WQs[2 * p + 1, sl, :]
            wk_p[:, et, p, 0:64] = WKh[2 * p, sl, :]
            wk_p[:, et, p, 64:128] = WKh[2 * p + 1, sl, :]

    wv_p = np.zeros((128, ET, 512), np.float16)
    wo_p = np.zeros((128, ET, 512), np.float16)
    for et in range(ET):
        sl = slice(et * 128, (et + 1) * 128)
        for h in range(H):
            wv_p[:, et, 64 * h : 64 * h + 64] = WVh[h, sl, :]
        wo_p[:, et, :] = WOh[:, sl].T

    bq_p = np.zeros((128, 4), np.float32)
    bk_p = np.zeros((128, 4), np.float32)
    for p in range(4):
        bq_p[0:64, p] = bQs[2 * p]
        bq_p[64:128, p] = bQs[2 * p + 1]
        bk_p[0:64, p] = np.asarray(bK, np.float32)[2 * p]
        bk_p[64:128, p] = np.asarray(bK, np.float32)[2 * p + 1]

    bv_p = np.asarray(bV, np.float32).T.copy()  # [64, H]
    bo_p = np.asarray(bO, np.float32).reshape(ET, 128).T.copy()  # [128, ET]

    return dict(
        wq=wq_p, wk=wk_p, wv=wv_p, wo=wo_p, bq=bq_p, bk=bk_p, bv=bv_p, bo=bo_p
    )


def _pack_seq_T(x2d):
    """[seq, E] f32 -> [128, ET, seq] f16 (transposed, E-tiled)."""
    seq = x2d.shape[0]
    return (
        np.ascontiguousarray(x2d.T)
        .reshape(ET, 128, seq)
        .transpose(1, 0, 2)
        .astype(np.float16)
    )


def prepare(embeddings_q, embeddings_k, embeddings_v, WQ, bQ, WK, bK, WV, bV, WO, bO):
    """Build per-core input maps. Returns in_maps list of dicts."""
    w = _pack_weights(WQ, bQ, WK, bK, WV, bV, WO, bO)
    eq = np.asarray(embeddings_q, np.float32)
    ek = np.asarray(embeddings_k, np.float32)
    ev = np.asarray(embeddings_v, np.float32)
    in_maps = []
    for b in range(B):
        xk_p = _pack_seq_T(ek[b])
        xv_p = _pack_seq_T(ev[b])
        for half in range(2):
            xq_p = _pack_seq_T(eq[b, half * NQ : (half + 1) * NQ, :])
            m = {"xq": xq_p, "xk": xk_p, "xv": xv_p}
            m.update(w)
            in_maps.append(m)
    return in_maps


def get_nc():
    if "nc" not in _CACHE:
        _CACHE["nc"] = _build_nc()
    return _CACHE["nc"]


def assemble(core_outs):
    """core_outs: list of 8 dicts with 'outT' [E, NQ] f32 -> [B, S, E]."""
    out = np.empty((B, S, E), np.float32)
    for b in range(B):
        for half in range(2):
            c = 2 * b + half
            out[b, half * NQ : (half + 1) * NQ, :] = core_outs[c]["outT"].T
    return out


def kernel(**inputs):
    from concourse.bass_utils import run_bass_kernel_spmd

    nc = get_nc()
    in_maps = prepare(**inputs)
    res = run_bass_kernel_spmd(nc, in_maps, core_ids=list(range(NCORES)))
    return assemble(res.results)
